# revision 18
# baseline (speedup 1.0000x reference)
"""Trainium2 Bass kernel for nn_DiffusionModule_67259187855466 (8 NeuronCores).

Sharding: sequence-parallel over the L=2048 rows (R=256 rows/core), weights
replicated, pair row-sharded. Per block, h1^T is AllGathered (bf16) and each
core recomputes the full K^T/V locally.

Pair bias: one streaming pass over the bf16 pair shard computes
bias[b,h,i,j] = pair[i,j,:] @ (sqrt(D)*pairW[b,h,:]) for all 4 blocks at once.
Pair tiles are transpose-DMA'd (c on partitions, two 64-c blocks stacked to
K=128, block-diagonal weights), cast to bf16 and spilled to DRAM in a
j-permuted order (all even j, then all odd j). K/V columns use the same
permutation, so softmax/attention results are unchanged.

Attention: scores in [j-tile(128) x i(256)] layout; PSUM gets q@k via matmul
plus the bias tile via an identity-matmul accumulate; ACT computes
exp(psum/sqrt(D)) directly (sqrt(D) is pre-folded into the bias weights; no
max-subtraction - logits are O(1) here). attn@V accumulates per head with an
extra ones-column in V producing the softmax denominator for free.
"""

import contextlib
import math
import sys

sys.path.insert(0, "/opt/trn_rl_repo")

import numpy as np
import ml_dtypes

import concourse.bass as bass
import concourse.mybir as mybir
import concourse.tile as tile
from concourse import bacc
from concourse import bass2jax

BF16 = mybir.dt.bfloat16
F32 = mybir.dt.float32
bf16 = ml_dtypes.bfloat16

L = 2048
NCORES = 8
R = L // NCORES          # 256
CA = 256
CS = 256
CZ = 64
H = 8
D = CA // H              # 32
NB = 4
FF = 4 * CA
VA = H * (D + 1)         # 264
SIGMA_DATA = 16.0
LN_EPS = 1e-5
INV_SQD = 1.0 / math.sqrt(D)

AFT = mybir.ActivationFunctionType
ALU = mybir.AluOpType
ts = bass.ts
ds = bass.ds


def _build_nc():
    nc = bacc.Bacc("TRN2", target_bir_lowering=False, debug=False,
                   enable_asserts=True, num_devices=NCORES)

    din = {}

    def inp(name, shape, dt):
        din[name] = nc.dram_tensor(name, shape, dt, kind="ExternalInput")

    inp("pairb", [R, 2, 512, 128], BF16)
    inp("xsct", [3, R], BF16)
    inp("xskip", [R, 4], F32)
    inp("snglt", [CS, R], BF16)
    inp("coordwt", [3, CA], BF16)
    inp("snglwt", [CS, CA], BF16)
    inp("h0brow", [1, CA], BF16)
    inp("qwt", [NB, CA, CA], BF16)
    inp("kwt", [NB, CA, CA], BF16)
    inp("vwt", [NB, CA, VA], BF16)
    inp("owt", [NB, CA, CA], BF16)
    inp("obrow", [NB, 1, CA], BF16)
    inp("w1t", [NB, CA, FF], BF16)
    inp("b1row", [NB, 1, FF], BF16)
    inp("w2t", [NB, FF, CA], BF16)
    inp("b2row", [NB, 1, CA], BF16)
    inp("pwbd", [128, CZ], BF16)
    inp("lnA", [NB, 2, CA], F32)
    inp("lnB", [NB, 2, CA], F32)
    inp("owf", [CA, 4], F32)
    inp("identb", [128, 128], BF16)
    inp("identf", [128, 128], F32)

    out = nc.dram_tensor("out", [R, 4], F32, kind="ExternalOutput")
    biasd = nc.dram_tensor("biasd", [NB * H, R, L], BF16, kind="Internal")
    agin = [nc.dram_tensor(f"agin{b}", [CA, R], BF16, kind="Internal")
            for b in range(NB)]
    agout = [nc.dram_tensor(f"agout{b}", [NCORES * CA, R], BF16,
                            kind="Internal", addr_space="Shared")
             for b in range(NB)]
    zdram = [nc.dram_tensor(f"zdram{b}", [H, R], F32, kind="Internal")
             for b in range(NB)]

    with tile.TileContext(nc) as tc:
        _body(nc, tc, din, out, biasd, agin, agout, zdram)

    nc.compile()
    return nc


def _body(nc, tc, din, out, biasd, agin, agout, zdram):
    ctx = contextlib.ExitStack()
    const = ctx.enter_context(tc.tile_pool(name="const", bufs=1))
    persist = ctx.enter_context(tc.tile_pool(name="persist", bufs=1))
    wpool = ctx.enter_context(tc.tile_pool(name="wpool", bufs=2))
    sb = ctx.enter_context(tc.tile_pool(name="sb", bufs=4))
    gsb = ctx.enter_context(tc.tile_pool(name="gsb", bufs=4))
    ps = ctx.enter_context(tc.tile_pool(name="ps", bufs=3, space="PSUM"))

    def dma(out_, in_, **kw):
        return nc.sync.dma_start(out=out_, in_=in_, **kw)

    def load(pool, src, tag, bufs=None):
        t = pool.tile(list(src.shape), src.dtype, tag=tag, bufs=bufs, name=tag)
        dma(t, src)
        return t

    ap = {k: v.ap() for k, v in din.items()}

    # ----- constants -----
    identb_t = load(const, ap["identb"], "identb")
    identf_t = load(const, ap["identf"], "identf")
    pwbd_t = load(const, ap["pwbd"], "pwbd")
    coordwt_t = load(const, ap["coordwt"], "coordwt")
    xsct_t = load(const, ap["xsct"], "xsct")
    h0brow_t = load(const, ap["h0brow"], "h0brow")
    ones1 = const.tile([1, 128], BF16, tag="ones1", name="ones1")
    nc.vector.memset(ones1, 1.0)
    onesR = const.tile([1, R], BF16, tag="onesR", name="onesR")
    nc.vector.memset(onesR, 1.0)
    eps_t = const.tile([128, 1], F32, tag="eps", name="eps")
    nc.vector.memset(eps_t, LN_EPS)
    snglt_t = [load(const, ap["snglt"][ts(ct, 128), :], f"snglt{ct}")
               for ct in range(2)]
    snglwt_t = [load(const, ap["snglwt"][ts(ct, 128), :], f"snglwt{ct}")
                for ct in range(2)]

    # ----- phase 0: pair-bias GEMM, all blocks at once -----
    for i0 in range(R):
        for q in range(2):
            rhs = gsb.tile([128, 512], BF16, tag="grhs", name="grhs")
            dma(rhs, ap["pairb"][i0, q], transpose=True)
            gps = ps.tile([64, 512], F32, tag="mps", name="gps")
            nc.tensor.matmul(gps, lhsT=pwbd_t, rhs=rhs, start=True, stop=True)
            cast = gsb.tile([64, 512], BF16, tag="gcast", name="gcast")
            if (2 * i0 + q) % 2 == 0:
                nc.scalar.activation(out=cast, in_=gps, func=AFT.Copy)
            else:
                nc.vector.tensor_copy(out=cast, in_=gps)
            # even j's -> pi cols [512q, +512); odd j's -> [1024+512q, +512)
            for eo in range(2):
                dst = bass.AP(tensor=biasd, offset=i0 * L + 1024 * eo + 512 * q,
                              ap=[[R * L, NB * H], [1, 512]])
                dma(dst, cast[ts(eo, 32), :])

    # ----- h0 -----
    h = []
    for ipt in range(2):
        hp = ps.tile([128, CA], F32, tag="mps", name="hps")
        isl = ts(ipt, 128)
        nc.tensor.matmul(hp, lhsT=snglt_t[0][:, isl], rhs=snglwt_t[0],
                         start=True, stop=False)
        nc.tensor.matmul(hp, lhsT=snglt_t[1][:, isl], rhs=snglwt_t[1],
                         start=False, stop=False)
        nc.tensor.matmul(hp, lhsT=xsct_t[:, isl], rhs=coordwt_t,
                         start=False, stop=False)
        nc.tensor.matmul(hp, lhsT=ones1, rhs=h0brow_t, start=False, stop=True)
        ht = persist.tile([128, CA], F32, tag=f"h{ipt}", name=f"h{ipt}")
        nc.vector.tensor_copy(out=ht, in_=hp)
        h.append(ht)

    # ----- blocks -----
    for b in range(NB):
        qwt_t = [load(wpool, ap["qwt"][b, ts(ct, 128), :], f"qwt{ct}")
                 for ct in range(2)]
        kwt_t = [load(wpool, ap["kwt"][b, ts(ct, 128), :], f"kwt{ct}")
                 for ct in range(2)]
        vwt_t = [load(wpool, ap["vwt"][b, ts(ct, 128), :], f"vwt{ct}")
                 for ct in range(2)]
        owt_t = [load(wpool, ap["owt"][b, ts(ct, 128), :], f"owt{ct}")
                 for ct in range(2)]
        w1t_t = [load(wpool, ap["w1t"][b, ts(ct, 128), :], f"w1t{ct}")
                 for ct in range(2)]
        w2t_t = [load(wpool, ap["w2t"][b, ts(ft, 128), :], f"w2t{ft}")
                 for ft in range(8)]
        obrow_t = load(wpool, ap["obrow"][b], "obrow")
        b1row_t = load(wpool, ap["b1row"][b], "b1row")
        b2row_t = load(wpool, ap["b2row"][b], "b2row")
        lnA_t = [load(wpool, bass.AP(tensor=din["lnA"], offset=(b * 2 + a) * CA,
                                     ap=[[0, 128], [1, CA]]), f"lnA{a}")
                 for a in range(2)]
        lnB_t = [load(wpool, bass.AP(tensor=din["lnB"], offset=(b * 2 + a) * CA,
                                     ap=[[0, 128], [1, CA]]), f"lnB{a}")
                 for a in range(2)]

        def adaln(a_idx, src):
            res = []
            for ipt in range(2):
                x = src[ipt]
                stats = sb.tile([128, 6], F32, tag="stats", name="stats")
                nc.vector.bn_stats(out=stats, in_=x)
                mv = sb.tile([128, 2], F32, tag="mv", name="mv")
                nc.vector.bn_aggr(out=mv, in_=stats)
                rstd = sb.tile([128, 1], F32, tag="rstd", name="rstd")
                nc.scalar.activation(out=rstd, in_=mv[:, 1:2], func=AFT.Sqrt,
                                     bias=eps_t)
                nc.vector.reciprocal(out=rstd, in_=rstd)
                xh = sb.tile([128, CA], F32, tag="xh", name="xh")
                nc.vector.tensor_scalar(out=xh, in0=x, scalar1=mv[:, 0:1],
                                        scalar2=rstd, op0=ALU.subtract,
                                        op1=ALU.mult)
                nc.vector.tensor_mul(out=xh, in0=xh, in1=lnA_t[a_idx])
                xb = sb.tile([128, CA], BF16, tag="xb", name="xb")
                nc.vector.tensor_add(out=xb, in0=xh, in1=lnB_t[a_idx])
                res.append(xb)
            return res

        def transpose_2x2(src, tagp):
            tt = [wpool.tile([128, 2, 128], BF16, tag=f"{tagp}{ct}",
                             name=f"{tagp}{ct}") for ct in range(2)]
            for ct in range(2):
                for ipt in range(2):
                    tp = ps.tile([128, 128], BF16, tag="mps", name="tps")
                    nc.tensor.transpose(tp, src[ipt][:, ts(ct, 128)], identb_t)
                    nc.vector.tensor_copy(out=tt[ct][:, ipt, :], in_=tp)
            return [t.rearrange("p a b -> p (a b)") for t in tt]

        # adaLN1 -> h1T -> AllGather
        h1 = adaln(0, h)
        h1T = transpose_2x2(h1, "h1T")
        for ct in range(2):
            dma(agin[b][ts(ct, 128), :], h1T[ct])
        nc.gpsimd.collective_compute(
            "AllGather", ALU.bypass,
            ins=[agin[b][:, :].opt()], outs=[agout[b][:, :].opt()],
            replica_groups=[list(range(NCORES))],
        )

        # qT / kT are stored as 3 tiles (heads 0-2, 3-5, 6-7) so that each
        # head's 32-partition slice starts at partition 0/32/64 (matmul
        # operands may not start at partition 96).
        def head_split_copy(dst3, psrc, hdt, colsl):
            if hdt == 0:
                nc.vector.tensor_copy(out=dst3[0][0:96, colsl],
                                      in_=psrc[0:96, :])
                nc.vector.tensor_copy(out=dst3[1][0:32, colsl],
                                      in_=psrc[96:128, :])
            else:
                # [32:96] would cross a 32-partition group boundary; split.
                nc.vector.tensor_copy(out=dst3[1][32:64, colsl],
                                      in_=psrc[0:32, :])
                nc.vector.tensor_copy(out=dst3[1][64:96, colsl],
                                      in_=psrc[32:64, :])
                nc.vector.tensor_copy(out=dst3[2][0:64, colsl],
                                      in_=psrc[64:128, :])

        qT = [persist.tile([96, R], BF16, tag=f"qT{t}", name=f"qT{t}")
              for t in range(2)] + \
             [persist.tile([64, R], BF16, tag="qT2", name="qT2")]
        for hdt in range(2):
            qp = ps.tile([128, R], F32, tag="mps", name="qps")
            for ct in range(2):
                nc.tensor.matmul(qp, lhsT=qwt_t[ct][:, ts(hdt, 128)],
                                 rhs=h1T[ct], start=(ct == 0), stop=(ct == 1))
            head_split_copy(qT, qp, hdt, slice(None))

        # full K^T and V_aug from the gathered h1T
        kT = [persist.tile([96, L], BF16, tag=f"kT{t}", name=f"kT{t}")
              for t in range(2)] + \
             [persist.tile([64, L], BF16, tag="kT2", name="kT2")]
        vA = [persist.tile([128, VA], BF16, tag=f"vA{t}", name=f"vA{t}")
              for t in range(16)]
        for r in range(NCORES):
            h1r = [gsb.tile([128, R], BF16, tag=f"h1r{ct}", name=f"h1r{ct}")
                   for ct in range(2)]
            for ct in range(2):
                dma(h1r[ct], agout[b][ds(r * CA + ct * 128, 128), :])
            for par in range(2):
                pi_t = par * 8 + r
                for hdt in range(2):
                    kp = ps.tile([128, 128], F32, tag="mps", name="kps")
                    for ct in range(2):
                        nc.tensor.matmul(
                            kp, lhsT=kwt_t[ct][:, ts(hdt, 128)],
                            rhs=h1r[ct][:, par::2],
                            start=(ct == 0), stop=(ct == 1))
                    head_split_copy(kT, kp, hdt, ts(pi_t, 128))
                vp = ps.tile([128, VA], F32, tag="mps", name="vps")
                for ct in range(2):
                    nc.tensor.matmul(vp, lhsT=h1r[ct][:, par::2],
                                     rhs=vwt_t[ct],
                                     start=(ct == 0), stop=(ct == 1))
                nc.vector.tensor_copy(out=vA[pi_t], in_=vp)
                nc.vector.memset(vA[pi_t][:, D::D + 1], 1.0)

        # attention: o and Z accumulate unnormalized; all heads' Z are
        # reciprocated together and partition-broadcast via a DRAM bounce.
        oTu = [wpool.tile([128, R], F32, tag=f"oTu{t}", name=f"oTu{t}")
               for t in range(2)]
        zall = sb.tile([1, H * R], F32, tag="zall", bufs=2, name="zall")
        for hh in range(H):
            htile, hsl = hh // 3, ds((hh % 3) * D, D)
            op = ps.tile([D + 1, R], F32, tag="ops", bufs=2, name="ops")
            for jt in range(16):
                sp = ps.tile([128, R], F32, tag="sps", bufs=3, name="sps")
                nc.tensor.matmul(sp, lhsT=kT[htile][hsl, ts(jt, 128)],
                                 rhs=qT[htile][hsl, :], start=True, stop=False)
                bt = sb.tile([128, R], BF16, tag="biast", name="biast")
                dma(bt, biasd.ap()[b * H + hh, :, ts(jt, 128)], transpose=True)
                nc.tensor.matmul(sp, lhsT=identb_t, rhs=bt,
                                 start=False, stop=True)
                es = sb.tile([128, R], BF16, tag="es", name="es")
                nc.scalar.activation(out=es, in_=sp, func=AFT.Exp,
                                     scale=INV_SQD)
                nc.tensor.matmul(op, lhsT=vA[jt][:, ds(hh * (D + 1), D + 1)],
                                 rhs=es, start=(jt == 0), stop=(jt == 15))
            nc.vector.tensor_copy(out=oTu[hh // 4][ds((hh % 4) * D, D), :],
                                  in_=op[0:D, :])
            nc.vector.reciprocal(out=zall[:, hh * R:(hh + 1) * R],
                                 in_=op[D:D + 1, :])
        dma(zdram[b].ap()[:, :], zall)
        oT = [wpool.tile([128, R], BF16, tag=f"oT{t}", name=f"oT{t}")
              for t in range(2)]
        for odt in range(2):
            rep = sb.tile([128, R], F32, tag="rep", bufs=2, name="rep")
            dma(rep, bass.AP(tensor=zdram[b], offset=odt * 4 * R,
                             ap=[[R, 4], [0, D], [1, R]]))
            nc.vector.tensor_mul(out=oT[odt], in0=oTu[odt], in1=rep)

        # out projection + residual
        for ipt in range(2):
            yp = ps.tile([128, CA], F32, tag="mps", name="yps")
            for hdt in range(2):
                nc.tensor.matmul(yp, lhsT=oT[hdt][:, ts(ipt, 128)],
                                 rhs=owt_t[hdt], start=(hdt == 0), stop=False)
            nc.tensor.matmul(yp, lhsT=ones1, rhs=obrow_t,
                             start=False, stop=True)
            nc.vector.tensor_add(out=h[ipt], in0=h[ipt], in1=yp)

        # FFN
        h2 = adaln(1, h)
        h2T = transpose_2x2(h2, "h2T")
        gT = [wpool.tile([128, R], BF16, tag=f"gT{ft}", name=f"gT{ft}")
              for ft in range(8)]
        for ft in range(8):
            up = ps.tile([128, R], F32, tag="mps", name="ups")
            for ct in range(2):
                nc.tensor.matmul(up, lhsT=w1t_t[ct][:, ts(ft, 128)],
                                 rhs=h2T[ct], start=(ct == 0), stop=False)
            nc.tensor.matmul(up, lhsT=b1row_t[:, ts(ft, 128)], rhs=onesR,
                             start=False, stop=True)
            nc.scalar.activation(out=gT[ft], in_=up, func=AFT.Gelu)
        for ipt in range(2):
            y2 = ps.tile([128, CA], F32, tag="mps", name="y2ps")
            for ft in range(8):
                nc.tensor.matmul(y2, lhsT=gT[ft][:, ts(ipt, 128)],
                                 rhs=w2t_t[ft], start=(ft == 0), stop=False)
            nc.tensor.matmul(y2, lhsT=ones1, rhs=b2row_t,
                             start=False, stop=True)
            nc.vector.tensor_add(out=h[ipt], in0=h[ipt], in1=y2)

    # ----- final projection -----
    owf_t = [load(const, ap["owf"][ts(ct, 128), :], f"owf{ct}")
             for ct in range(2)]
    xskip_t = [load(const, ap["xskip"][ts(ipt, 128), :], f"xskip{ipt}")
               for ipt in range(2)]
    hT = [sb.tile([128, 2, 128], F32, tag=f"hT{ct}", name=f"hT{ct}")
          for ct in range(2)]
    for ct in range(2):
        for ipt in range(2):
            tp = ps.tile([128, 128], F32, tag="mps", name="tpsf")
            nc.tensor.transpose(tp, h[ipt][:, ts(ct, 128)], identf_t)
            nc.vector.tensor_copy(out=hT[ct][:, ipt, :], in_=tp)
    hTm = [t.rearrange("p a b -> p (a b)") for t in hT]
    for ipt in range(2):
        fp = ps.tile([128, 4], F32, tag="mps", name="fps")
        for ct in range(2):
            nc.tensor.matmul(fp, lhsT=hTm[ct][:, ts(ipt, 128)],
                             rhs=owf_t[ct], start=(ct == 0), stop=(ct == 1))
        ot = sb.tile([128, 4], F32, tag="ot", name="ot")
        nc.vector.tensor_add(out=ot, in0=fp, in1=xskip_t[ipt])
        dma(out.ap()[ts(ipt, 128), :], ot)
    ctx.close()


# ------------------------------------------------------------------
# host side
# ------------------------------------------------------------------
_CACHE = {}


def _gelu_np(x):
    try:
        from scipy.special import erf
        e = erf(x / math.sqrt(2.0))
    except Exception:
        e = np.vectorize(math.erf)(x / math.sqrt(2.0))
    return 0.5 * x * (1.0 + e)


def _host_prep(inputs):
    f32 = np.float32
    sigma = float(inputs["sigma"])
    sd = SIGMA_DATA
    s2 = sigma * sigma + sd * sd
    c_skip = f32(sd * sd / s2)
    c_out = f32(sigma * sd / math.sqrt(s2))
    c_in = f32(1.0 / math.sqrt(s2))
    c_noise = f32(0.25 * math.log(sigma + 1e-8))

    half = CA // 2
    freqs = np.exp(-math.log(10000.0)
                   * np.arange(half, dtype=f32) / half).astype(f32)
    a = c_noise * freqs
    temb = np.concatenate([np.cos(a), np.sin(a)]).astype(f32)
    t1 = _gelu_np((temb @ inputs["tmlp_W1"].T
                   + inputs["tmlp_b1"]).astype(np.float64))
    tc_vec = (t1 @ inputs["tmlp_W2"].T.astype(np.float64)
              + inputs["tmlp_b2"]).astype(f32)

    lnA = np.zeros((NB, 2, CA), f32)
    lnB = np.zeros((NB, 2, CA), f32)
    for b in range(NB):
        for a_i, (g, bb, pW, pb) in enumerate([
            (inputs["ada1_g"][b], inputs["ada1_b"][b],
             inputs["ada1_pW"][b], inputs["ada1_pb"][b]),
            (inputs["ada2_g"][b], inputs["ada2_b"][b],
             inputs["ada2_pW"][b], inputs["ada2_pb"][b]),
        ]):
            ss = tc_vec @ pW.T + pb
            scale, shift = ss[:CA], ss[CA:]
            lnA[b, a_i] = g * (1.0 + scale)
            lnB[b, a_i] = bb * (1.0 + scale) + shift

    vw_aug = np.zeros((NB, VA, CA), f32)
    for hh in range(H):
        vw_aug[:, hh * (D + 1):hh * (D + 1) + D, :] = \
            inputs["vW"][:, hh * D:(hh + 1) * D, :]

    wall = (math.sqrt(D) * inputs["pairW"].reshape(NB * H, CZ)).astype(f32)
    pwbd = np.zeros((128, CZ), f32)
    pwbd[0:CZ, 0:32] = wall.T
    pwbd[CZ:128, 32:64] = wall.T

    pair_bf = np.ascontiguousarray(inputs["pair"]).astype(bf16)
    pair_bf = pair_bf.reshape(NCORES, R, 2, 512, 128)

    x = inputs["x_noisy"].astype(f32)
    xskip = np.zeros((L, 4), f32)
    xskip[:, 0:3] = c_skip * x + c_out * inputs["out_b"][None, :]
    owf = np.zeros((CA, 4), f32)
    owf[:, 0:3] = c_out * inputs["out_W"].T

    shared = {
        "coordwt": np.ascontiguousarray(inputs["coord_W"].T).astype(bf16),
        "snglwt": np.ascontiguousarray(inputs["single_W"].T).astype(bf16),
        "h0brow": (inputs["coord_b"]
                   + inputs["single_b"]).reshape(1, CA).astype(bf16),
        "qwt": np.ascontiguousarray(inputs["qW"].transpose(0, 2, 1)).astype(bf16),
        "kwt": np.ascontiguousarray(inputs["kW"].transpose(0, 2, 1)).astype(bf16),
        "vwt": np.ascontiguousarray(vw_aug.transpose(0, 2, 1)).astype(bf16),
        "owt": np.ascontiguousarray(inputs["outW"].transpose(0, 2, 1)).astype(bf16),
        "obrow": inputs["outb"].reshape(NB, 1, CA).astype(bf16),
        "w1t": np.ascontiguousarray(inputs["ffn_W1"].transpose(0, 2, 1)).astype(bf16),
        "b1row": inputs["ffn_b1"].reshape(NB, 1, FF).astype(bf16),
        "w2t": np.ascontiguousarray(inputs["ffn_W2"].transpose(0, 2, 1)).astype(bf16),
        "b2row": inputs["ffn_b2"].reshape(NB, 1, CA).astype(bf16),
        "pwbd": pwbd.astype(bf16),
        "lnA": lnA, "lnB": lnB, "owf": owf,
        "identb": np.eye(128, dtype=f32).astype(bf16),
        "identf": np.eye(128, dtype=f32),
    }
    xct = np.ascontiguousarray((c_in * x).T).astype(bf16)
    sngl = inputs["single"].astype(f32)

    in_maps = []
    for c in range(NCORES):
        rows = slice(c * R, (c + 1) * R)
        m = dict(shared)
        m["pairb"] = pair_bf[c]
        m["xsct"] = np.ascontiguousarray(xct[:, rows])
        m["xskip"] = np.ascontiguousarray(xskip[rows])
        m["snglt"] = np.ascontiguousarray(sngl[rows].T).astype(bf16)
        in_maps.append(m)
    return in_maps


def _get_nc():
    if "nc" not in _CACHE:
        _CACHE["nc"] = _build_nc()
    return _CACHE["nc"]


def _get_exec():
    """Cached jitted 8-core executor (mirrors bass2jax.run_bass_via_pjrt)."""
    if "exec" in _CACHE:
        return _CACHE["exec"]
    import jax
    import jax.numpy as jnp
    from jax.experimental.shard_map import shard_map
    from jax.sharding import Mesh, PartitionSpec
    from concourse.bass2jax import (_bass_exec_p, install_neuronx_cc_hook,
                                    partition_id_tensor)
    import concourse.mybir as mb

    nc = _get_nc()
    install_neuronx_cc_hook()
    pname = nc.partition_id_tensor.name if nc.partition_id_tensor else None
    in_names, out_names, out_avals, zero_shapes = [], [], [], []
    for alloc in nc.m.functions[0].allocations:
        if not isinstance(alloc, mb.MemoryLocationSet):
            continue
        name = alloc.memorylocations[0].name
        if alloc.kind == "ExternalInput":
            if name != pname:
                in_names.append(name)
        elif alloc.kind == "ExternalOutput":
            shape = tuple(alloc.tensor_shape)
            dtype = mb.dt.np(alloc.dtype)
            out_names.append(name)
            out_avals.append(jax.core.ShapedArray(shape, dtype))
            zero_shapes.append((shape, dtype))
    n_params = len(in_names)
    all_names = in_names + out_names
    if pname is not None:
        all_names = all_names + [pname]
    donate = tuple(range(n_params, n_params + len(out_names)))

    def _bodyfn(*args):
        operands = list(args)
        if pname is not None:
            operands.append(partition_id_tensor())
        outs = _bass_exec_p.bind(
            *operands, out_avals=tuple(out_avals), in_names=tuple(all_names),
            out_names=tuple(out_names), lowering_input_output_aliases=(),
            sim_require_finite=True, sim_require_nnan=True, nc=nc)
        return tuple(outs)

    devices = jax.devices()[:NCORES]
    mesh = Mesh(np.asarray(devices), ("core",))
    specs = (PartitionSpec("core"),) * (n_params + len(out_names))
    sharded = jax.jit(
        shard_map(_bodyfn, mesh=mesh, in_specs=specs,
                  out_specs=(PartitionSpec("core"),) * len(out_names),
                  check_rep=False),
        donate_argnums=donate, keep_unused=True)
    _CACHE["exec"] = dict(fn=sharded, in_names=in_names, out_names=out_names,
                          zero_shapes=zero_shapes, mesh=mesh)
    return _CACHE["exec"]


def _run(in_maps):
    ex = _get_exec()
    concat_in = [np.concatenate([np.asarray(m[n]) for m in in_maps], axis=0)
                 for n in ex["in_names"]]
    zeros = [np.zeros((NCORES * s[0], *s[1:]), d) for s, d in ex["zero_shapes"]]
    outs = ex["fn"](*concat_in, *zeros)
    return outs


def kernel(**inputs):
    inputs = {k: np.asarray(v) for k, v in inputs.items()}
    in_maps = _host_prep(inputs)
    outs = _run(in_maps)
    oi = _get_exec()["out_names"].index("out")
    full = np.asarray(outs[oi]).reshape(NCORES, R, 4)
    return np.ascontiguousarray(
        full[:, :, 0:3].reshape(L, 3)).astype(np.float32)


def bench(in_maps, iters=10):
    """Wall-clock the cached executor with device-resident inputs."""
    import time
    import jax
    from jax.sharding import NamedSharding, PartitionSpec
    ex = _get_exec()
    sh = NamedSharding(ex["mesh"], PartitionSpec("core"))
    concat_in = [jax.device_put(
        np.concatenate([np.asarray(m[n]) for m in in_maps], axis=0), sh)
        for n in ex["in_names"]]
    for a in concat_in:
        a.block_until_ready()
    times = []
    for _ in range(iters):
        zeros = [jax.device_put(np.zeros((NCORES * s[0], *s[1:]), d), sh)
                 for s, d in ex["zero_shapes"]]
        for z in zeros:
            z.block_until_ready()
        t0 = time.perf_counter()
        outs = ex["fn"](*concat_in, *zeros)
        for o in outs:
            o.block_until_ready()
        times.append(time.perf_counter() - t0)
    return times, outs


if __name__ == "__main__":
    import reference
    ins = {k: np.asarray(v) for k, v in reference.setup_inputs().items()}
    got = kernel(**ins)
    want = np.asarray(reference.reference(**reference.setup_inputs()))
    rel = np.linalg.norm(got - want) / np.linalg.norm(want)
    print("max abs err", np.abs(got - want).max(), "rel l2", rel)


# revision 19
# speedup vs baseline: 27.0058x; 27.0058x over previous
"""Trainium2 Bass kernel for nn_DiffusionModule_67259187855466 (8 NeuronCores).

Sharding: sequence-parallel over the L=2048 rows (R=256 rows/core), weights
replicated, pair row-sharded. Per block, h1^T is AllGathered (bf16) and each
core recomputes the full K^T/V locally.

Pair bias: one streaming pass over the bf16 pair shard computes
bias[b,h,i,j] = pair[i,j,:] @ (sqrt(D)*pairW[b,h,:]) for all 4 blocks at once.
Pair tiles are transpose-DMA'd (c on partitions, two 64-c blocks stacked to
K=128, block-diagonal weights), cast to bf16 and spilled to DRAM in a
j-permuted order (all even j, then all odd j). K/V columns use the same
permutation, so softmax/attention results are unchanged.

Attention: scores in [j-tile(128) x i(256)] layout; PSUM gets q@k via matmul
plus the bias tile via an identity-matmul accumulate; ACT computes
exp(psum/sqrt(D)) directly (sqrt(D) is pre-folded into the bias weights; no
max-subtraction - logits are O(1) here). attn@V accumulates per head with an
extra ones-column in V producing the softmax denominator for free.
"""

import contextlib
import math
import sys

sys.path.insert(0, "/opt/trn_rl_repo")

import numpy as np
import ml_dtypes

import concourse.bass as bass
import concourse.mybir as mybir
import concourse.tile as tile
from concourse import bacc
from concourse import bass2jax

BF16 = mybir.dt.bfloat16
F32 = mybir.dt.float32
bf16 = ml_dtypes.bfloat16

L = 2048
NCORES = 8
R = L // NCORES          # 256
CA = 256
CS = 256
CZ = 64
H = 8
D = CA // H              # 32
NB = 4
FF = 4 * CA
VA = H * (D + 1)         # 264
SIGMA_DATA = 16.0
LN_EPS = 1e-5
INV_SQD = 1.0 / math.sqrt(D)

AFT = mybir.ActivationFunctionType
ALU = mybir.AluOpType
ts = bass.ts
ds = bass.ds


def _build_nc():
    nc = bacc.Bacc("TRN2", target_bir_lowering=False, debug=False,
                   enable_asserts=True, num_devices=NCORES)

    din = {}

    def inp(name, shape, dt):
        din[name] = nc.dram_tensor(name, shape, dt, kind="ExternalInput")

    inp("pairb", [R, 2, 512, 128], BF16)
    inp("xsct", [3, R], BF16)
    inp("xskip", [R, 4], F32)
    inp("snglt", [CS, R], BF16)
    inp("coordwt", [3, CA], BF16)
    inp("snglwt", [CS, CA], BF16)
    inp("h0brow", [1, CA], BF16)
    inp("qwt", [NB, CA, CA], BF16)
    inp("kwt", [NB, CA, CA], BF16)
    inp("vwt", [NB, CA, VA], BF16)
    inp("owt", [NB, CA, CA], BF16)
    inp("obrow", [NB, 1, CA], BF16)
    inp("w1t", [NB, CA, FF], BF16)
    inp("b1row", [NB, 1, FF], BF16)
    inp("w2t", [NB, FF, CA], BF16)
    inp("b2row", [NB, 1, CA], BF16)
    inp("pwbd", [128, CZ], BF16)
    inp("lnA", [NB, 2, CA], F32)
    inp("lnB", [NB, 2, CA], F32)
    inp("owf", [CA, 4], F32)
    inp("identb", [128, 128], BF16)
    inp("identf", [128, 128], F32)

    out = nc.dram_tensor("out", [R, 4], F32, kind="ExternalOutput")
    biasd = nc.dram_tensor("biasd", [NB * H, R, L], BF16, kind="Internal")
    agin = [nc.dram_tensor(f"agin{b}", [CA, R], BF16, kind="Internal")
            for b in range(NB)]
    agout = [nc.dram_tensor(f"agout{b}", [NCORES * CA, R], BF16,
                            kind="Internal", addr_space="Shared")
             for b in range(NB)]
    zdram = [nc.dram_tensor(f"zdram{b}", [H, R], F32, kind="Internal")
             for b in range(NB)]

    with tile.TileContext(nc) as tc:
        _body(nc, tc, din, out, biasd, agin, agout, zdram)

    nc.compile()
    return nc


def _body(nc, tc, din, out, biasd, agin, agout, zdram):
    ctx = contextlib.ExitStack()
    const = ctx.enter_context(tc.tile_pool(name="const", bufs=1))
    persist = ctx.enter_context(tc.tile_pool(name="persist", bufs=1))
    wpool = ctx.enter_context(tc.tile_pool(name="wpool", bufs=2))
    sb = ctx.enter_context(tc.tile_pool(name="sb", bufs=4))
    gsb = ctx.enter_context(tc.tile_pool(name="gsb", bufs=4))
    ps = ctx.enter_context(tc.tile_pool(name="ps", bufs=3, space="PSUM"))

    def dma(out_, in_, **kw):
        return nc.sync.dma_start(out=out_, in_=in_, **kw)

    def load(pool, src, tag, bufs=None):
        t = pool.tile(list(src.shape), src.dtype, tag=tag, bufs=bufs, name=tag)
        dma(t, src)
        return t

    ap = {k: v.ap() for k, v in din.items()}

    # ----- constants -----
    identb_t = load(const, ap["identb"], "identb")
    identf_t = load(const, ap["identf"], "identf")
    pwbd_t = load(const, ap["pwbd"], "pwbd")
    coordwt_t = load(const, ap["coordwt"], "coordwt")
    xsct_t = load(const, ap["xsct"], "xsct")
    h0brow_t = load(const, ap["h0brow"], "h0brow")
    ones1 = const.tile([1, 128], BF16, tag="ones1", name="ones1")
    nc.vector.memset(ones1, 1.0)
    onesR = const.tile([1, R], BF16, tag="onesR", name="onesR")
    nc.vector.memset(onesR, 1.0)
    eps_t = const.tile([128, 1], F32, tag="eps", name="eps")
    nc.vector.memset(eps_t, LN_EPS)
    snglt_t = [load(const, ap["snglt"][ts(ct, 128), :], f"snglt{ct}")
               for ct in range(2)]
    snglwt_t = [load(const, ap["snglwt"][ts(ct, 128), :], f"snglwt{ct}")
                for ct in range(2)]

    # ----- phase 0: pair-bias GEMM, all blocks at once -----
    for i0 in range(R):
        for q in range(2):
            rhs = gsb.tile([128, 512], BF16, tag="grhs", name="grhs")
            dma(rhs, ap["pairb"][i0, q], transpose=True)
            gps = ps.tile([64, 512], F32, tag="mps", name="gps")
            nc.tensor.matmul(gps, lhsT=pwbd_t, rhs=rhs, start=True, stop=True)
            cast = gsb.tile([64, 512], BF16, tag="gcast", name="gcast")
            if (2 * i0 + q) % 2 == 0:
                nc.scalar.activation(out=cast, in_=gps, func=AFT.Copy)
            else:
                nc.vector.tensor_copy(out=cast, in_=gps)
            # even j's -> pi cols [512q, +512); odd j's -> [1024+512q, +512)
            for eo in range(2):
                dst = bass.AP(tensor=biasd, offset=i0 * L + 1024 * eo + 512 * q,
                              ap=[[R * L, NB * H], [1, 512]])
                dma(dst, cast[ts(eo, 32), :])

    # ----- h0 -----
    h = []
    for ipt in range(2):
        hp = ps.tile([128, CA], F32, tag="mps", name="hps")
        isl = ts(ipt, 128)
        nc.tensor.matmul(hp, lhsT=snglt_t[0][:, isl], rhs=snglwt_t[0],
                         start=True, stop=False)
        nc.tensor.matmul(hp, lhsT=snglt_t[1][:, isl], rhs=snglwt_t[1],
                         start=False, stop=False)
        nc.tensor.matmul(hp, lhsT=xsct_t[:, isl], rhs=coordwt_t,
                         start=False, stop=False)
        nc.tensor.matmul(hp, lhsT=ones1, rhs=h0brow_t, start=False, stop=True)
        ht = persist.tile([128, CA], F32, tag=f"h{ipt}", name=f"h{ipt}")
        nc.vector.tensor_copy(out=ht, in_=hp)
        h.append(ht)

    # ----- blocks -----
    for b in range(NB):
        qwt_t = [load(wpool, ap["qwt"][b, ts(ct, 128), :], f"qwt{ct}")
                 for ct in range(2)]
        kwt_t = [load(wpool, ap["kwt"][b, ts(ct, 128), :], f"kwt{ct}")
                 for ct in range(2)]
        vwt_t = [load(wpool, ap["vwt"][b, ts(ct, 128), :], f"vwt{ct}")
                 for ct in range(2)]
        owt_t = [load(wpool, ap["owt"][b, ts(ct, 128), :], f"owt{ct}")
                 for ct in range(2)]
        w1t_t = [load(wpool, ap["w1t"][b, ts(ct, 128), :], f"w1t{ct}")
                 for ct in range(2)]
        w2t_t = [load(wpool, ap["w2t"][b, ts(ft, 128), :], f"w2t{ft}")
                 for ft in range(8)]
        obrow_t = load(wpool, ap["obrow"][b], "obrow")
        b1row_t = load(wpool, ap["b1row"][b], "b1row")
        b2row_t = load(wpool, ap["b2row"][b], "b2row")
        lnA_t = [load(wpool, bass.AP(tensor=din["lnA"], offset=(b * 2 + a) * CA,
                                     ap=[[0, 128], [1, CA]]), f"lnA{a}")
                 for a in range(2)]
        lnB_t = [load(wpool, bass.AP(tensor=din["lnB"], offset=(b * 2 + a) * CA,
                                     ap=[[0, 128], [1, CA]]), f"lnB{a}")
                 for a in range(2)]

        def adaln(a_idx, src):
            res = []
            for ipt in range(2):
                x = src[ipt]
                stats = sb.tile([128, 6], F32, tag="stats", name="stats")
                nc.vector.bn_stats(out=stats, in_=x)
                mv = sb.tile([128, 2], F32, tag="mv", name="mv")
                nc.vector.bn_aggr(out=mv, in_=stats)
                rstd = sb.tile([128, 1], F32, tag="rstd", name="rstd")
                nc.scalar.activation(out=rstd, in_=mv[:, 1:2], func=AFT.Sqrt,
                                     bias=eps_t)
                nc.vector.reciprocal(out=rstd, in_=rstd)
                xh = sb.tile([128, CA], F32, tag="xh", name="xh")
                nc.vector.tensor_scalar(out=xh, in0=x, scalar1=mv[:, 0:1],
                                        scalar2=rstd, op0=ALU.subtract,
                                        op1=ALU.mult)
                nc.vector.tensor_mul(out=xh, in0=xh, in1=lnA_t[a_idx])
                xb = sb.tile([128, CA], BF16, tag="xb", name="xb")
                nc.vector.tensor_add(out=xb, in0=xh, in1=lnB_t[a_idx])
                res.append(xb)
            return res

        def transpose_2x2(src, tagp):
            tt = [wpool.tile([128, 2, 128], BF16, tag=f"{tagp}{ct}",
                             name=f"{tagp}{ct}") for ct in range(2)]
            for ct in range(2):
                for ipt in range(2):
                    tp = ps.tile([128, 128], BF16, tag="mps", name="tps")
                    nc.tensor.transpose(tp, src[ipt][:, ts(ct, 128)], identb_t)
                    nc.vector.tensor_copy(out=tt[ct][:, ipt, :], in_=tp)
            return [t.rearrange("p a b -> p (a b)") for t in tt]

        # adaLN1 -> h1T -> AllGather
        h1 = adaln(0, h)
        h1T = transpose_2x2(h1, "h1T")
        for ct in range(2):
            dma(agin[b][ts(ct, 128), :], h1T[ct])
        nc.gpsimd.collective_compute(
            "AllGather", ALU.bypass,
            ins=[agin[b][:, :].opt()], outs=[agout[b][:, :].opt()],
            replica_groups=[list(range(NCORES))],
        )

        # qT / kT are stored as 3 tiles (heads 0-2, 3-5, 6-7) so that each
        # head's 32-partition slice starts at partition 0/32/64 (matmul
        # operands may not start at partition 96).
        def head_split_copy(dst3, psrc, hdt, colsl):
            if hdt == 0:
                nc.vector.tensor_copy(out=dst3[0][0:96, colsl],
                                      in_=psrc[0:96, :])
                nc.vector.tensor_copy(out=dst3[1][0:32, colsl],
                                      in_=psrc[96:128, :])
            else:
                # [32:96] would cross a 32-partition group boundary; split.
                nc.vector.tensor_copy(out=dst3[1][32:64, colsl],
                                      in_=psrc[0:32, :])
                nc.vector.tensor_copy(out=dst3[1][64:96, colsl],
                                      in_=psrc[32:64, :])
                nc.vector.tensor_copy(out=dst3[2][0:64, colsl],
                                      in_=psrc[64:128, :])

        qT = [persist.tile([96, R], BF16, tag=f"qT{t}", name=f"qT{t}")
              for t in range(2)] + \
             [persist.tile([64, R], BF16, tag="qT2", name="qT2")]
        for hdt in range(2):
            qp = ps.tile([128, R], F32, tag="mps", name="qps")
            for ct in range(2):
                nc.tensor.matmul(qp, lhsT=qwt_t[ct][:, ts(hdt, 128)],
                                 rhs=h1T[ct], start=(ct == 0), stop=(ct == 1))
            head_split_copy(qT, qp, hdt, slice(None))

        # full K^T and V_aug from the gathered h1T
        kT = [persist.tile([96, L], BF16, tag=f"kT{t}", name=f"kT{t}")
              for t in range(2)] + \
             [persist.tile([64, L], BF16, tag="kT2", name="kT2")]
        vA = [persist.tile([128, VA], BF16, tag=f"vA{t}", name=f"vA{t}")
              for t in range(16)]
        for r in range(NCORES):
            h1r = [gsb.tile([128, R], BF16, tag=f"h1r{ct}", name=f"h1r{ct}")
                   for ct in range(2)]
            for ct in range(2):
                dma(h1r[ct], agout[b][ds(r * CA + ct * 128, 128), :])
            for par in range(2):
                pi_t = par * 8 + r
                for hdt in range(2):
                    kp = ps.tile([128, 128], F32, tag="mps", name="kps")
                    for ct in range(2):
                        nc.tensor.matmul(
                            kp, lhsT=kwt_t[ct][:, ts(hdt, 128)],
                            rhs=h1r[ct][:, par::2],
                            start=(ct == 0), stop=(ct == 1))
                    head_split_copy(kT, kp, hdt, ts(pi_t, 128))
                vp = ps.tile([128, VA], F32, tag="mps", name="vps")
                for ct in range(2):
                    nc.tensor.matmul(vp, lhsT=h1r[ct][:, par::2],
                                     rhs=vwt_t[ct],
                                     start=(ct == 0), stop=(ct == 1))
                nc.vector.tensor_copy(out=vA[pi_t], in_=vp)
                nc.vector.memset(vA[pi_t][:, D::D + 1], 1.0)

        # attention: o and Z accumulate unnormalized; all heads' Z are
        # reciprocated together and partition-broadcast via a DRAM bounce.
        oTu = [wpool.tile([128, R], F32, tag=f"oTu{t}", name=f"oTu{t}")
               for t in range(2)]
        zall = sb.tile([1, H * R], F32, tag="zall", bufs=2, name="zall")
        for hh in range(H):
            htile, hsl = hh // 3, ds((hh % 3) * D, D)
            op = ps.tile([D + 1, R], F32, tag="ops", bufs=2, name="ops")
            for jt in range(16):
                sp = ps.tile([128, R], F32, tag="sps", bufs=3, name="sps")
                nc.tensor.matmul(sp, lhsT=kT[htile][hsl, ts(jt, 128)],
                                 rhs=qT[htile][hsl, :], start=True, stop=False)
                bt = sb.tile([128, R], BF16, tag="biast", name="biast")
                dma(bt, biasd.ap()[b * H + hh, :, ts(jt, 128)], transpose=True)
                nc.tensor.matmul(sp, lhsT=identb_t, rhs=bt,
                                 start=False, stop=True)
                es = sb.tile([128, R], BF16, tag="es", name="es")
                nc.scalar.activation(out=es, in_=sp, func=AFT.Exp,
                                     scale=INV_SQD)
                nc.tensor.matmul(op, lhsT=vA[jt][:, ds(hh * (D + 1), D + 1)],
                                 rhs=es, start=(jt == 0), stop=(jt == 15))
            nc.vector.tensor_copy(out=oTu[hh // 4][ds((hh % 4) * D, D), :],
                                  in_=op[0:D, :])
            nc.vector.reciprocal(out=zall[:, hh * R:(hh + 1) * R],
                                 in_=op[D:D + 1, :])
        dma(zdram[b].ap()[:, :], zall)
        oT = [wpool.tile([128, R], BF16, tag=f"oT{t}", name=f"oT{t}")
              for t in range(2)]
        for odt in range(2):
            rep = sb.tile([128, R], F32, tag="rep", bufs=2, name="rep")
            dma(rep, bass.AP(tensor=zdram[b], offset=odt * 4 * R,
                             ap=[[R, 4], [0, D], [1, R]]))
            nc.vector.tensor_mul(out=oT[odt], in0=oTu[odt], in1=rep)

        # out projection + residual
        for ipt in range(2):
            yp = ps.tile([128, CA], F32, tag="mps", name="yps")
            for hdt in range(2):
                nc.tensor.matmul(yp, lhsT=oT[hdt][:, ts(ipt, 128)],
                                 rhs=owt_t[hdt], start=(hdt == 0), stop=False)
            nc.tensor.matmul(yp, lhsT=ones1, rhs=obrow_t,
                             start=False, stop=True)
            nc.vector.tensor_add(out=h[ipt], in0=h[ipt], in1=yp)

        # FFN
        h2 = adaln(1, h)
        h2T = transpose_2x2(h2, "h2T")
        gT = [wpool.tile([128, R], BF16, tag=f"gT{ft}", name=f"gT{ft}")
              for ft in range(8)]
        for ft in range(8):
            up = ps.tile([128, R], F32, tag="mps", name="ups")
            for ct in range(2):
                nc.tensor.matmul(up, lhsT=w1t_t[ct][:, ts(ft, 128)],
                                 rhs=h2T[ct], start=(ct == 0), stop=False)
            nc.tensor.matmul(up, lhsT=b1row_t[:, ts(ft, 128)], rhs=onesR,
                             start=False, stop=True)
            nc.scalar.activation(out=gT[ft], in_=up, func=AFT.Gelu)
        for ipt in range(2):
            y2 = ps.tile([128, CA], F32, tag="mps", name="y2ps")
            for ft in range(8):
                nc.tensor.matmul(y2, lhsT=gT[ft][:, ts(ipt, 128)],
                                 rhs=w2t_t[ft], start=(ft == 0), stop=False)
            nc.tensor.matmul(y2, lhsT=ones1, rhs=b2row_t,
                             start=False, stop=True)
            nc.vector.tensor_add(out=h[ipt], in0=h[ipt], in1=y2)

    # ----- final projection -----
    owf_t = [load(const, ap["owf"][ts(ct, 128), :], f"owf{ct}")
             for ct in range(2)]
    xskip_t = [load(const, ap["xskip"][ts(ipt, 128), :], f"xskip{ipt}")
               for ipt in range(2)]
    hT = [sb.tile([128, 2, 128], F32, tag=f"hT{ct}", name=f"hT{ct}")
          for ct in range(2)]
    for ct in range(2):
        for ipt in range(2):
            tp = ps.tile([128, 128], F32, tag="mps", name="tpsf")
            nc.tensor.transpose(tp, h[ipt][:, ts(ct, 128)], identf_t)
            nc.vector.tensor_copy(out=hT[ct][:, ipt, :], in_=tp)
    hTm = [t.rearrange("p a b -> p (a b)") for t in hT]
    for ipt in range(2):
        fp = ps.tile([128, 4], F32, tag="mps", name="fps")
        for ct in range(2):
            nc.tensor.matmul(fp, lhsT=hTm[ct][:, ts(ipt, 128)],
                             rhs=owf_t[ct], start=(ct == 0), stop=(ct == 1))
        ot = sb.tile([128, 4], F32, tag="ot", name="ot")
        nc.vector.tensor_add(out=ot, in0=fp, in1=xskip_t[ipt])
        dma(out.ap()[ts(ipt, 128), :], ot)
    ctx.close()


# ------------------------------------------------------------------
# host side
# ------------------------------------------------------------------
_CACHE = {}


def _gelu_np(x):
    try:
        from scipy.special import erf
        e = erf(x / math.sqrt(2.0))
    except Exception:
        e = np.vectorize(math.erf)(x / math.sqrt(2.0))
    return 0.5 * x * (1.0 + e)


def _host_prep(inputs):
    f32 = np.float32
    sigma = float(inputs["sigma"])
    sd = SIGMA_DATA
    s2 = sigma * sigma + sd * sd
    c_skip = f32(sd * sd / s2)
    c_out = f32(sigma * sd / math.sqrt(s2))
    c_in = f32(1.0 / math.sqrt(s2))
    c_noise = f32(0.25 * math.log(sigma + 1e-8))

    half = CA // 2
    freqs = np.exp(-math.log(10000.0)
                   * np.arange(half, dtype=f32) / half).astype(f32)
    a = c_noise * freqs
    temb = np.concatenate([np.cos(a), np.sin(a)]).astype(f32)
    t1 = _gelu_np((temb @ inputs["tmlp_W1"].T
                   + inputs["tmlp_b1"]).astype(np.float64))
    tc_vec = (t1 @ inputs["tmlp_W2"].T.astype(np.float64)
              + inputs["tmlp_b2"]).astype(f32)

    lnA = np.zeros((NB, 2, CA), f32)
    lnB = np.zeros((NB, 2, CA), f32)
    for b in range(NB):
        for a_i, (g, bb, pW, pb) in enumerate([
            (inputs["ada1_g"][b], inputs["ada1_b"][b],
             inputs["ada1_pW"][b], inputs["ada1_pb"][b]),
            (inputs["ada2_g"][b], inputs["ada2_b"][b],
             inputs["ada2_pW"][b], inputs["ada2_pb"][b]),
        ]):
            ss = tc_vec @ pW.T + pb
            scale, shift = ss[:CA], ss[CA:]
            lnA[b, a_i] = g * (1.0 + scale)
            lnB[b, a_i] = bb * (1.0 + scale) + shift

    vw_aug = np.zeros((NB, VA, CA), f32)
    for hh in range(H):
        vw_aug[:, hh * (D + 1):hh * (D + 1) + D, :] = \
            inputs["vW"][:, hh * D:(hh + 1) * D, :]

    wall = (math.sqrt(D) * inputs["pairW"].reshape(NB * H, CZ)).astype(f32)
    pwbd = np.zeros((128, CZ), f32)
    pwbd[0:CZ, 0:32] = wall.T
    pwbd[CZ:128, 32:64] = wall.T

    pair_bf = np.ascontiguousarray(inputs["pair"]).astype(bf16)
    pair_bf = pair_bf.reshape(NCORES, R, 2, 512, 128)

    x = inputs["x_noisy"].astype(f32)
    xskip = np.zeros((L, 4), f32)
    xskip[:, 0:3] = c_skip * x + c_out * inputs["out_b"][None, :]
    owf = np.zeros((CA, 4), f32)
    owf[:, 0:3] = c_out * inputs["out_W"].T

    shared = {
        "coordwt": np.ascontiguousarray(inputs["coord_W"].T).astype(bf16),
        "snglwt": np.ascontiguousarray(inputs["single_W"].T).astype(bf16),
        "h0brow": (inputs["coord_b"]
                   + inputs["single_b"]).reshape(1, CA).astype(bf16),
        "qwt": np.ascontiguousarray(inputs["qW"].transpose(0, 2, 1)).astype(bf16),
        "kwt": np.ascontiguousarray(inputs["kW"].transpose(0, 2, 1)).astype(bf16),
        "vwt": np.ascontiguousarray(vw_aug.transpose(0, 2, 1)).astype(bf16),
        "owt": np.ascontiguousarray(inputs["outW"].transpose(0, 2, 1)).astype(bf16),
        "obrow": inputs["outb"].reshape(NB, 1, CA).astype(bf16),
        "w1t": np.ascontiguousarray(inputs["ffn_W1"].transpose(0, 2, 1)).astype(bf16),
        "b1row": inputs["ffn_b1"].reshape(NB, 1, FF).astype(bf16),
        "w2t": np.ascontiguousarray(inputs["ffn_W2"].transpose(0, 2, 1)).astype(bf16),
        "b2row": inputs["ffn_b2"].reshape(NB, 1, CA).astype(bf16),
        "pwbd": pwbd.astype(bf16),
        "lnA": lnA, "lnB": lnB, "owf": owf,
        "identb": np.eye(128, dtype=f32).astype(bf16),
        "identf": np.eye(128, dtype=f32),
    }
    xct = np.ascontiguousarray((c_in * x).T).astype(bf16)
    sngl = inputs["single"].astype(f32)

    in_maps = []
    for c in range(NCORES):
        rows = slice(c * R, (c + 1) * R)
        m = dict(shared)
        m["pairb"] = pair_bf[c]
        m["xsct"] = np.ascontiguousarray(xct[:, rows])
        m["xskip"] = np.ascontiguousarray(xskip[rows])
        m["snglt"] = np.ascontiguousarray(sngl[rows].T).astype(bf16)
        in_maps.append(m)
    return in_maps


def _get_nc():
    if "nc" not in _CACHE:
        _CACHE["nc"] = _build_nc()
    return _CACHE["nc"]


def _get_exec():
    """Cached jitted 8-core executor (mirrors bass2jax.run_bass_via_pjrt)."""
    if "exec" in _CACHE:
        return _CACHE["exec"]
    import jax
    import jax.numpy as jnp
    from jax.experimental.shard_map import shard_map
    from jax.sharding import Mesh, PartitionSpec
    from concourse.bass2jax import (_bass_exec_p, install_neuronx_cc_hook,
                                    partition_id_tensor)
    import concourse.mybir as mb

    nc = _get_nc()
    install_neuronx_cc_hook()
    pname = nc.partition_id_tensor.name if nc.partition_id_tensor else None
    in_names, out_names, out_avals, zero_shapes = [], [], [], []
    for alloc in nc.m.functions[0].allocations:
        if not isinstance(alloc, mb.MemoryLocationSet):
            continue
        name = alloc.memorylocations[0].name
        if alloc.kind == "ExternalInput":
            if name != pname:
                in_names.append(name)
        elif alloc.kind == "ExternalOutput":
            shape = tuple(alloc.tensor_shape)
            dtype = mb.dt.np(alloc.dtype)
            out_names.append(name)
            out_avals.append(jax.core.ShapedArray(shape, dtype))
            zero_shapes.append((shape, dtype))
    n_params = len(in_names)
    all_names = in_names + out_names
    if pname is not None:
        all_names = all_names + [pname]
    donate = tuple(range(n_params, n_params + len(out_names)))

    def _bodyfn(*args):
        operands = list(args)
        if pname is not None:
            operands.append(partition_id_tensor())
        outs = _bass_exec_p.bind(
            *operands, out_avals=tuple(out_avals), in_names=tuple(all_names),
            out_names=tuple(out_names), lowering_input_output_aliases=(),
            sim_require_finite=True, sim_require_nnan=True, nc=nc)
        return tuple(outs)

    devices = jax.devices()[:NCORES]
    mesh = Mesh(np.asarray(devices), ("core",))
    specs = (PartitionSpec("core"),) * (n_params + len(out_names))
    sharded = jax.jit(
        shard_map(_bodyfn, mesh=mesh, in_specs=specs,
                  out_specs=(PartitionSpec("core"),) * len(out_names),
                  check_rep=False),
        donate_argnums=donate, keep_unused=True)
    _CACHE["exec"] = dict(fn=sharded, in_names=in_names, out_names=out_names,
                          zero_shapes=zero_shapes, mesh=mesh)
    return _CACHE["exec"]


def _run(in_maps):
    ex = _get_exec()
    concat_in = [np.concatenate([np.asarray(m[n]) for m in in_maps], axis=0)
                 for n in ex["in_names"]]
    zeros = [np.zeros((NCORES * s[0], *s[1:]), d) for s, d in ex["zero_shapes"]]
    outs = ex["fn"](*concat_in, *zeros)
    return outs


def kernel(**inputs):
    inputs = {k: np.asarray(v) for k, v in inputs.items()}
    in_maps = _host_prep(inputs)
    outs = _run(in_maps)
    oi = _get_exec()["out_names"].index("out")
    full = np.asarray(outs[oi]).reshape(NCORES, R, 4)
    return np.ascontiguousarray(
        full[:, :, 0:3].reshape(L, 3)).astype(np.float32)


def bench(in_maps, iters=10):
    """Wall-clock the cached executor with device-resident inputs."""
    import time
    import jax
    from jax.sharding import NamedSharding, PartitionSpec
    ex = _get_exec()
    sh = NamedSharding(ex["mesh"], PartitionSpec("core"))
    concat_in = [jax.device_put(
        np.concatenate([np.asarray(m[n]) for m in in_maps], axis=0), sh)
        for n in ex["in_names"]]
    for a in concat_in:
        a.block_until_ready()
    times = []
    for _ in range(iters):
        zeros = [jax.device_put(np.zeros((NCORES * s[0], *s[1:]), d), sh)
                 for s, d in ex["zero_shapes"]]
        for z in zeros:
            z.block_until_ready()
        t0 = time.perf_counter()
        outs = ex["fn"](*concat_in, *zeros)
        for o in outs:
            o.block_until_ready()
        times.append(time.perf_counter() - t0)
    return times, outs


def bench_slope(in_maps, n_small=2, n_big=22):
    """Marginal per-dispatch time: fire N async dispatches, block once.
    slope = (t_big - t_small) / (n_big - n_small) ~= device time + per-call
    axon marshaling (~0.5 ms floor)."""
    import time
    import jax
    from jax.sharding import NamedSharding, PartitionSpec
    ex = _get_exec()
    sh = NamedSharding(ex["mesh"], PartitionSpec("core"))
    concat_in = [jax.device_put(
        np.concatenate([np.asarray(m[n]) for m in in_maps], axis=0), sh)
        for n in ex["in_names"]]
    for a in concat_in:
        a.block_until_ready()

    def run_n(n):
        zs = [[jax.device_put(np.zeros((NCORES * s[0], *s[1:]), d), sh)
               for s, d in ex["zero_shapes"]] for _ in range(n)]
        for z in zs:
            for x in z:
                x.block_until_ready()
        t0 = time.perf_counter()
        outs = None
        for i in range(n):
            outs = ex["fn"](*concat_in, *zs[i])
        for o in outs:
            o.block_until_ready()
        return time.perf_counter() - t0

    run_n(2)  # warm
    ts = run_n(n_small)
    tb = run_n(n_big)
    return ts, tb, (tb - ts) / (n_big - n_small)


if __name__ == "__main__":
    import reference
    ins = {k: np.asarray(v) for k, v in reference.setup_inputs().items()}
    got = kernel(**ins)
    want = np.asarray(reference.reference(**reference.setup_inputs()))
    rel = np.linalg.norm(got - want) / np.linalg.norm(want)
    print("max abs err", np.abs(got - want).max(), "rel l2", rel)


# revision 49
# speedup vs baseline: 28.5099x; 1.0557x over previous
"""Trainium2 Bass kernel for nn_DiffusionModule_67259187855466 (8 NeuronCores).

Sharding: sequence-parallel over the L=2048 rows (R=256 rows/core), weights
replicated, pair row-sharded. Per block, h1^T is AllGathered (bf16) and each
core recomputes the full K^T/V locally.

Pair bias: one streaming pass over the bf16 pair shard computes
bias[b,h,i,j] = pair[i,j,:] @ (sqrt(D)*pairW[b,h,:]) for all 4 blocks at once.
Pair tiles are transpose-DMA'd (c on partitions, two 64-c blocks stacked to
K=128, block-diagonal weights), cast to bf16 and spilled to DRAM in a
j-permuted order (all even j, then all odd j). K/V columns use the same
permutation, so softmax/attention results are unchanged.

Attention: scores in [j-tile(128) x i(256)] layout; PSUM gets q@k via matmul
plus the bias tile via an identity-matmul accumulate; ACT computes
exp(psum/sqrt(D)) directly (sqrt(D) is pre-folded into the bias weights; no
max-subtraction - logits are O(1) here). attn@V accumulates per head with an
extra ones-column in V producing the softmax denominator for free.
"""

import contextlib
import math
import sys

sys.path.insert(0, "/opt/trn_rl_repo")

import numpy as np
import ml_dtypes

import concourse.bass as bass
import concourse.mybir as mybir
import concourse.tile as tile
from concourse import bacc
from concourse import bass2jax

BF16 = mybir.dt.bfloat16
F32 = mybir.dt.float32
bf16 = ml_dtypes.bfloat16

L = 2048
NCORES = 8
R = L // NCORES          # 256
CA = 256
CS = 256
CZ = 64
H = 8
D = CA // H              # 32
NB = 4
FF = 4 * CA
VA = H * (D + 1)         # 264
SIGMA_DATA = 16.0
LN_EPS = 1e-5
INV_SQD = 1.0 / math.sqrt(D)

AFT = mybir.ActivationFunctionType
ALU = mybir.AluOpType
ts = bass.ts
ds = bass.ds


def _build_nc(phase0=True, blocks=NB, local_ag=False, final=True,
              compile=True, attn_mode="old_newbias"):
    nc = bacc.Bacc("TRN2", target_bir_lowering=False, debug=False,
                   enable_asserts=True, num_devices=NCORES)

    din = {}

    def inp(name, shape, dt):
        din[name] = nc.dram_tensor(name, shape, dt, kind="ExternalInput")

    inp("pairb", [R, 2, 512, 128], BF16)
    inp("xsct", [3, R], BF16)
    inp("xskip", [R, 4], F32)
    inp("snglt", [CS, R], BF16)
    inp("coordwt", [3, CA], BF16)
    inp("snglwt", [CS, CA], BF16)
    inp("h0brow", [1, CA], BF16)
    inp("qwt", [NB, CA, CA], BF16)
    inp("kwt", [NB, CA, CA], BF16)
    inp("vwt", [NB, CA, VA], BF16)
    inp("owt", [NB, CA, CA], BF16)
    inp("w1t", [NB, CA, FF], BF16)
    inp("w2t", [NB, FF, CA], BF16)
    inp("brows", [NB, 1, CA + FF + CA], BF16)  # outb | ffn_b1 | ffn_b2
    inp("pwbd", [128, CZ], BF16)
    inp("lnA", [NB, 2, CA], F32)
    inp("lnB", [NB, 2, CA], F32)
    inp("owf", [CA, 4], F32)
    inp("identb", [128, 128], BF16)
    inp("identf", [128, 128], F32)

    out = nc.dram_tensor("out", [R, 4], F32, kind="ExternalOutput")
    biasd = nc.dram_tensor("biasd", [NB * H, R, L], BF16, kind="Internal")
    agin = [nc.dram_tensor(f"agin{b}", [CA, R], BF16, kind="Internal")
            for b in range(NB)]
    agout = [nc.dram_tensor(f"agout{b}", [NCORES * CA, R], BF16,
                            kind="Internal", addr_space="Shared")
             for b in range(NB)]
    zdram = [nc.dram_tensor(f"zdram{b}", [H, R], F32, kind="Internal")
             for b in range(NB)]

    with tile.TileContext(nc) as tc:
        _body(nc, tc, din, out, biasd, agin, agout, zdram,
              phase0=phase0, blocks=blocks, local_ag=local_ag, final=final,
              attn_mode=attn_mode)

    if compile:
        nc.compile()
    return nc


def _body(nc, tc, din, out, biasd, agin, agout, zdram,
          phase0=True, blocks=NB, local_ag=False, final=True,
          attn_mode="new"):
    ctx = contextlib.ExitStack()
    const = ctx.enter_context(tc.tile_pool(name="const", bufs=1))
    persist = ctx.enter_context(tc.tile_pool(name="persist", bufs=1))
    wpool = ctx.enter_context(tc.tile_pool(name="wpool", bufs=2))
    sb = ctx.enter_context(tc.tile_pool(name="sb", bufs=4))
    gsb = ctx.enter_context(tc.tile_pool(name="gsb", bufs=4))
    bpool = ctx.enter_context(tc.tile_pool(name="bpool", bufs=1))
    ps = ctx.enter_context(tc.tile_pool(name="ps", bufs=2, space="PSUM"))

    def dma(out_, in_, **kw):
        return nc.sync.dma_start(out=out_, in_=in_, **kw)

    def load(pool, src, tag, bufs=None):
        t = pool.tile(list(src.shape), src.dtype, tag=tag, bufs=bufs, name=tag)
        dma(t, src)
        return t

    ap = {k: v.ap() for k, v in din.items()}

    # ----- constants -----
    identb_t = load(const, ap["identb"], "identb")
    identf_t = load(const, ap["identf"], "identf")
    pwbd_t = load(const, ap["pwbd"], "pwbd")
    coordwt_t = load(const, ap["coordwt"], "coordwt")
    xsct_t = load(const, ap["xsct"], "xsct")
    h0brow_t = load(const, ap["h0brow"], "h0brow")
    ones1 = const.tile([1, 128], BF16, tag="ones1", name="ones1")
    nc.vector.memset(ones1, 1.0)
    onesR = const.tile([1, R], BF16, tag="onesR", name="onesR")
    nc.vector.memset(onesR, 1.0)
    eps_t = const.tile([128, 1], F32, tag="eps", name="eps")
    nc.vector.memset(eps_t, LN_EPS)
    snglt_t = [load(const, ap["snglt"][ts(ct, 128), :], f"snglt{ct}")
               for ct in range(2)]
    snglwt_t = [load(const, ap["snglwt"][ts(ct, 128), :], f"snglwt{ct}")
                for ct in range(2)]

    # ----- phase 0: pair-bias GEMM, all blocks at once -----
    # 2 i-rows per transpose-DMA; per i-row, 2 matmuls [64, 512] whose
    # columns n are j-pairs: psum rows 0:32 even j (pi 512q+n), 32:64 odd
    # (pi 1024+512q+n). Both q psums cast into one [64, 1024] bf16 tile,
    # spilled with a single 4D-AP DMA.
    for ii in range(0, R if phase0 else 0, 2):
        rhs = gsb.tile([128, 2048], BF16, tag="grhs", bufs=2, name="grhs")
        src = bass.AP(tensor=din["pairb"], offset=ii * L * CZ,
                      ap=[[128, 2048], [1, 128]])
        nc.scalar.dma_start(out=rhs, in_=src, transpose=True)
        for d_i in range(2):
            cast = gsb.tile([64, 1024], BF16, tag="gcast", bufs=3, name="gcast")
            for q in range(2):
                gps = ps.tile([64, 512], F32, tag="gps", bufs=2, name="gps")
                nc.tensor.matmul(gps, lhsT=pwbd_t,
                                 rhs=rhs[:, ds(d_i * 1024 + q * 512, 512)],
                                 start=True, stop=True)
                if q == 0:
                    nc.scalar.activation(out=cast[:, 0:512], in_=gps,
                                         func=AFT.Copy)
                else:
                    nc.vector.tensor_copy(out=cast[:, 512:1024], in_=gps)
            dst = bass.AP(tensor=biasd, offset=(ii + d_i) * L,
                          ap=[[1024, 2], [R * L, NB * H], [512, 2], [1, 512]])
            dma(dst, cast[:, :])

    # ----- h0 -----
    h = []
    for ipt in range(2):
        hp = ps.tile([128, CA], F32, tag="mps", name="hps")
        isl = ts(ipt, 128)
        nc.tensor.matmul(hp, lhsT=snglt_t[0][:, isl], rhs=snglwt_t[0],
                         start=True, stop=False)
        nc.tensor.matmul(hp, lhsT=snglt_t[1][:, isl], rhs=snglwt_t[1],
                         start=False, stop=False)
        nc.tensor.matmul(hp, lhsT=xsct_t[:, isl], rhs=coordwt_t,
                         start=False, stop=False)
        nc.tensor.matmul(hp, lhsT=ones1, rhs=h0brow_t, start=False, stop=True)
        ht = persist.tile([128, CA], F32, tag=f"h{ipt}", name=f"h{ipt}")
        nc.vector.tensor_copy(out=ht, in_=hp)
        h.append(ht)

    # ----- blocks -----
    for b in range(blocks):
        def loadw(name, nt, width):
            src = bass.AP(tensor=din[name], offset=b * 128 * nt * width,
                          ap=[[width, 128], [width * 128, nt], [1, width]])
            return load(wpool, src, name, bufs=1)

        qwt_m = loadw("qwt", 2, CA)
        kwt_m = loadw("kwt", 2, CA)
        vwt_m = loadw("vwt", 2, VA)
        owt_m = loadw("owt", 2, CA)
        w1t_m = loadw("w1t", 2, FF)
        w2t_m = loadw("w2t", 8, CA)
        qwt_t = [qwt_m[:, ct, :] for ct in range(2)]
        kwt_t = [kwt_m[:, ct, :] for ct in range(2)]
        vwt_t = [vwt_m[:, ct, :] for ct in range(2)]
        owt_t = [owt_m[:, ct, :] for ct in range(2)]
        w1t_t = [w1t_m[:, ct, :] for ct in range(2)]
        w2t_t = [w2t_m[:, ft, :] for ft in range(8)]
        brows_t = load(wpool, ap["brows"][b], "brows", bufs=1)
        obrow_t = brows_t[:, 0:CA]
        b1row_t = brows_t[:, CA:CA + FF]
        b2row_t = brows_t[:, CA + FF:CA + FF + CA]
        lnA_m = load(wpool, bass.AP(tensor=din["lnA"], offset=b * 2 * CA,
                                    ap=[[0, 128], [CA, 2], [1, CA]]),
                     "lnA", bufs=1)
        lnB_m = load(wpool, bass.AP(tensor=din["lnB"], offset=b * 2 * CA,
                                    ap=[[0, 128], [CA, 2], [1, CA]]),
                     "lnB", bufs=1)
        lnA_t = [lnA_m[:, a, :] for a in range(2)]
        lnB_t = [lnB_m[:, a, :] for a in range(2)]

        def adaln(a_idx, src):
            res = []
            for ipt in range(2):
                x = src[ipt]
                stats = sb.tile([128, 6], F32, tag="stats", name="stats")
                nc.vector.bn_stats(out=stats, in_=x)
                mv = sb.tile([128, 2], F32, tag="mv", name="mv")
                nc.vector.bn_aggr(out=mv, in_=stats)
                rstd = sb.tile([128, 1], F32, tag="rstd", name="rstd")
                nc.scalar.activation(out=rstd, in_=mv[:, 1:2], func=AFT.Sqrt,
                                     bias=eps_t)
                nc.vector.reciprocal(out=rstd, in_=rstd)
                xh = sb.tile([128, CA], F32, tag="xh", name="xh")
                nc.vector.tensor_scalar(out=xh, in0=x, scalar1=mv[:, 0:1],
                                        scalar2=rstd, op0=ALU.subtract,
                                        op1=ALU.mult)
                nc.vector.tensor_mul(out=xh, in0=xh, in1=lnA_t[a_idx])
                xb = sb.tile([128, CA], BF16, tag="xb", name="xb")
                nc.vector.tensor_add(out=xb, in0=xh, in1=lnB_t[a_idx])
                res.append(xb)
            return res

        def transpose_2x2(src, tagp):
            tt = [wpool.tile([128, 2, 128], BF16, tag=f"{tagp}{ct}",
                             name=f"{tagp}{ct}") for ct in range(2)]
            for ct in range(2):
                for ipt in range(2):
                    tp = ps.tile([128, 128], BF16, tag="mps", name="tps")
                    nc.tensor.transpose(tp, src[ipt][:, ts(ct, 128)], identb_t)
                    nc.vector.tensor_copy(out=tt[ct][:, ipt, :], in_=tp)
            return [t.rearrange("p a b -> p (a b)") for t in tt]

        # adaLN1 -> h1T -> AllGather
        h1 = adaln(0, h)
        h1T = transpose_2x2(h1, "h1T")
        for ct in range(2):
            dma(agin[b][ts(ct, 128), :], h1T[ct])
        if local_ag:
            for r in range(NCORES):
                dma(agout[b][ds(r * CA, CA), :], agin[b][:, :])
        else:
            nc.gpsimd.collective_compute(
                "AllGather", ALU.bypass,
                ins=[agin[b][:, :].opt()], outs=[agout[b][:, :].opt()],
                replica_groups=[list(range(NCORES))],
            )

        # qT / kT are stored as 3 tiles (heads 0-2, 3-5, 6-7) so that each
        # head's 32-partition slice starts at partition 0/32/64 (matmul
        # operands may not start at partition 96).
        def head_split_copy(dst3, psrc, hdt, colsl):
            if hdt == 0:
                nc.vector.tensor_copy(out=dst3[0][0:96, colsl],
                                      in_=psrc[0:96, :])
                nc.vector.tensor_copy(out=dst3[1][0:32, colsl],
                                      in_=psrc[96:128, :])
            else:
                # [32:96] would cross a 32-partition group boundary; split.
                nc.vector.tensor_copy(out=dst3[1][32:64, colsl],
                                      in_=psrc[0:32, :])
                nc.vector.tensor_copy(out=dst3[1][64:96, colsl],
                                      in_=psrc[32:64, :])
                nc.vector.tensor_copy(out=dst3[2][0:64, colsl],
                                      in_=psrc[64:128, :])

        qT = [persist.tile([96, R], BF16, tag=f"qT{t}", name=f"qT{t}")
              for t in range(2)] + \
             [persist.tile([64, R], BF16, tag="qT2", name="qT2")]
        for hdt in range(2):
            qp = ps.tile([128, R], F32, tag="mps", name="qps")
            for ct in range(2):
                nc.tensor.matmul(qp, lhsT=qwt_t[ct][:, ts(hdt, 128)],
                                 rhs=h1T[ct], start=(ct == 0), stop=(ct == 1))
            head_split_copy(qT, qp, hdt, slice(None))

        # full K^T and V_aug from the gathered h1T
        kT = [persist.tile([96, L], BF16, tag=f"kT{t}", name=f"kT{t}")
              for t in range(2)] + \
             [persist.tile([64, L], BF16, tag="kT2", name="kT2")]
        vA = [persist.tile([128, VA], BF16, tag=f"vA{t}", name=f"vA{t}")
              for t in range(16)]
        for r in range(NCORES):
            h1r = [gsb.tile([128, R], BF16, tag=f"h1r{ct}", name=f"h1r{ct}")
                   for ct in range(2)]
            for ct in range(2):
                dma(h1r[ct], agout[b][ds(r * CA + ct * 128, 128), :])
            for par in range(2):
                pi_t = par * 8 + r
                for hdt in range(2):
                    kp = ps.tile([128, 128], F32, tag="mps", name="kps")
                    for ct in range(2):
                        nc.tensor.matmul(
                            kp, lhsT=kwt_t[ct][:, ts(hdt, 128)],
                            rhs=h1r[ct][:, par::2],
                            start=(ct == 0), stop=(ct == 1))
                    head_split_copy(kT, kp, hdt, ts(pi_t, 128))
                vp = ps.tile([128, VA], F32, tag="mps", name="vps")
                for ct in range(2):
                    nc.tensor.matmul(vp, lhsT=h1r[ct][:, par::2],
                                     rhs=vwt_t[ct],
                                     start=(ct == 0), stop=(ct == 1))
                nc.vector.tensor_copy(out=vA[pi_t], in_=vp)
                nc.vector.memset(vA[pi_t][:, D::D + 1], 1.0)

        oTu = [wpool.tile([128, R], F32, tag=f"oTu{t}", name=f"oTu{t}")
               for t in range(2)]

        def attn_new(merge=True):
            # bias tiles for all 8 heads of this block, resident per jt:
            # one transpose-DMA loads [2048(=8h x 256i), 128j] -> [128j, 2048]
            # (consecutive bh rows of biasd are contiguous so (h, i) merges).
            biasG = [bpool.tile([128, H * R], BF16, tag=f"biasG{jt}",
                                name=f"biasG{jt}") for jt in range(16)]
            for jt in range(16):
                src = bass.AP(tensor=biasd, offset=b * H * R * L + jt * 128,
                              ap=[[L, H * R], [1, 128]])
                nc.scalar.dma_start(out=biasG[jt], in_=src, transpose=True)
            # head pairs share one [128, 512] score psum: bias preload via
            # identity-matmul, qk accumulates, one exp per pair.
            for hp in range(4):
                h0, h1 = 2 * hp, 2 * hp + 1
                ops_pair = [ps.tile([D + 1, R], F32, tag="ops", bufs=2,
                                    name="ops") for _ in range(2)]
                for jt in range(16):
                    if merge:
                        sp = ps.tile([128, 2 * R], F32, tag="sps", bufs=2,
                                     name="sps")
                        nc.tensor.matmul(sp, lhsT=identb_t,
                                         rhs=biasG[jt][:, ds(h0 * R, 2 * R)],
                                         start=True, stop=False)
                        for ii_h, hh in enumerate((h0, h1)):
                            htile, hsl = hh // 3, ds((hh % 3) * D, D)
                            nc.tensor.matmul(sp[:, ds(ii_h * R, R)],
                                             lhsT=kT[htile][hsl, ts(jt, 128)],
                                             rhs=qT[htile][hsl, :],
                                             start=False, stop=(ii_h == 1))
                        es = sb.tile([128, 2 * R], BF16, tag="es", name="es")
                        nc.scalar.activation(out=es, in_=sp, func=AFT.Exp,
                                             scale=INV_SQD)
                        epair = [es[:, 0:R], es[:, R:2 * R]]
                    else:
                        epair = []
                        for ii_h, hh in enumerate((h0, h1)):
                            htile, hsl = hh // 3, ds((hh % 3) * D, D)
                            sp = ps.tile([128, R], F32, tag="sps", bufs=2,
                                         name="sps")
                            nc.tensor.matmul(sp, lhsT=identb_t,
                                             rhs=biasG[jt][:, ds(hh * R, R)],
                                             start=True, stop=False)
                            nc.tensor.matmul(sp,
                                             lhsT=kT[htile][hsl, ts(jt, 128)],
                                             rhs=qT[htile][hsl, :],
                                             start=False, stop=True)
                            es = sb.tile([128, R], BF16, tag="es", name="es")
                            nc.scalar.activation(out=es, in_=sp, func=AFT.Exp,
                                                 scale=INV_SQD)
                            epair.append(es)
                    for ii_h, hh in enumerate((h0, h1)):
                        nc.tensor.matmul(
                            ops_pair[ii_h],
                            lhsT=vA[jt][:, ds(hh * (D + 1), D + 1)],
                            rhs=epair[ii_h],
                            start=(jt == 0), stop=(jt == 15))
                zpair = sb.tile([1, 2 * R], F32, tag="zpair", bufs=2,
                                name="zpair")
                for ii_h, hh in enumerate((h0, h1)):
                    op = ops_pair[ii_h]
                    nc.vector.tensor_copy(
                        out=oTu[hh // 4][ds((hh % 4) * D, D), :],
                        in_=op[0:D, :])
                    nc.vector.reciprocal(out=zpair[:, ds(ii_h * R, R)],
                                         in_=op[D:D + 1, :])
                dma(bass.AP(tensor=zdram[b], offset=h0 * R, ap=[[1, 2 * R]]),
                    zpair)

        def attn_old(newbias=False):
            if newbias:
                biasG = [bpool.tile([128, H * R], BF16, tag=f"biasG{jt}",
                                    name=f"biasG{jt}") for jt in range(16)]
                for jt in range(16):
                    src = bass.AP(tensor=biasd,
                                  offset=b * H * R * L + jt * 128,
                                  ap=[[L, H * R], [1, 128]])
                    nc.scalar.dma_start(out=biasG[jt], in_=src,
                                        transpose=True)
            for hh in range(H):
                htile, hsl = hh // 3, ds((hh % 3) * D, D)
                op = ps.tile([D + 1, R], F32, tag="ops", bufs=2, name="ops")
                for jt in range(16):
                    sp = ps.tile([128, R], F32, tag="sps", bufs=2, name="sps")
                    nc.tensor.matmul(sp, lhsT=kT[htile][hsl, ts(jt, 128)],
                                     rhs=qT[htile][hsl, :],
                                     start=True, stop=False)
                    if newbias:
                        bt = biasG[jt][:, ds(hh * R, R)]
                    else:
                        bt = sb.tile([128, R], BF16, tag="biast", name="biast")
                        nc.scalar.dma_start(
                            out=bt, in_=biasd.ap()[b * H + hh, :, ts(jt, 128)],
                            transpose=True)
                    nc.tensor.matmul(sp, lhsT=identb_t, rhs=bt,
                                     start=False, stop=True)
                    es = sb.tile([128, R], BF16, tag="es", name="es")
                    nc.scalar.activation(out=es, in_=sp, func=AFT.Exp,
                                         scale=INV_SQD)
                    nc.tensor.matmul(
                        op, lhsT=vA[jt][:, ds(hh * (D + 1), D + 1)],
                        rhs=es, start=(jt == 0), stop=(jt == 15))
                zpair = sb.tile([1, R], F32, tag="zpair", bufs=2, name="zpair")
                nc.vector.tensor_copy(out=oTu[hh // 4][ds((hh % 4) * D, D), :],
                                      in_=op[0:D, :])
                nc.vector.reciprocal(out=zpair, in_=op[D:D + 1, :])
                dma(bass.AP(tensor=zdram[b], offset=hh * R, ap=[[1, R]]),
                    zpair)

        if attn_mode == "old":
            attn_old()
        elif attn_mode == "old_newbias":
            attn_old(newbias=True)
        elif attn_mode == "new_sep":
            attn_new(merge=False)
        else:
            attn_new()
        oT = [wpool.tile([128, R], BF16, tag=f"oT{t}", name=f"oT{t}")
              for t in range(2)]
        for odt in range(2):
            rep = sb.tile([128, R], F32, tag="rep", bufs=2, name="rep")
            dma(rep, bass.AP(tensor=zdram[b], offset=odt * 4 * R,
                             ap=[[R, 4], [0, D], [1, R]]))
            nc.vector.tensor_mul(out=oT[odt], in0=oTu[odt], in1=rep)

        # out projection + residual
        for ipt in range(2):
            yp = ps.tile([128, CA], F32, tag="mps", name="yps")
            for hdt in range(2):
                nc.tensor.matmul(yp, lhsT=oT[hdt][:, ts(ipt, 128)],
                                 rhs=owt_t[hdt], start=(hdt == 0), stop=False)
            nc.tensor.matmul(yp, lhsT=ones1, rhs=obrow_t,
                             start=False, stop=True)
            nc.vector.tensor_add(out=h[ipt], in0=h[ipt], in1=yp)

        # FFN
        h2 = adaln(1, h)
        h2T = transpose_2x2(h2, "h2T")
        gT = [wpool.tile([128, R], BF16, tag=f"gT{ft}", name=f"gT{ft}")
              for ft in range(8)]
        for ft in range(8):
            up = ps.tile([128, R], F32, tag="mps", name="ups")
            for ct in range(2):
                nc.tensor.matmul(up, lhsT=w1t_t[ct][:, ts(ft, 128)],
                                 rhs=h2T[ct], start=(ct == 0), stop=False)
            nc.tensor.matmul(up, lhsT=b1row_t[:, ts(ft, 128)], rhs=onesR,
                             start=False, stop=True)
            nc.scalar.activation(out=gT[ft], in_=up, func=AFT.Gelu)
        for ipt in range(2):
            y2 = ps.tile([128, CA], F32, tag="mps", name="y2ps")
            for ft in range(8):
                nc.tensor.matmul(y2, lhsT=gT[ft][:, ts(ipt, 128)],
                                 rhs=w2t_t[ft], start=(ft == 0), stop=False)
            nc.tensor.matmul(y2, lhsT=ones1, rhs=b2row_t,
                             start=False, stop=True)
            nc.vector.tensor_add(out=h[ipt], in0=h[ipt], in1=y2)

    # ----- final projection -----
    if not final:
        for ipt in range(2):
            ot = sb.tile([128, 4], F32, tag="ot", name="ot")
            nc.vector.tensor_copy(out=ot, in_=h[ipt][:, 0:4])
            dma(out.ap()[ts(ipt, 128), :], ot)
        ctx.close()
        return
    owf_t = [load(const, ap["owf"][ts(ct, 128), :], f"owf{ct}")
             for ct in range(2)]
    xskip_t = [load(const, ap["xskip"][ts(ipt, 128), :], f"xskip{ipt}")
               for ipt in range(2)]
    hT = [sb.tile([128, 2, 128], F32, tag=f"hT{ct}", name=f"hT{ct}")
          for ct in range(2)]
    for ct in range(2):
        for ipt in range(2):
            tp = ps.tile([128, 128], F32, tag="mps", name="tpsf")
            nc.tensor.transpose(tp, h[ipt][:, ts(ct, 128)], identf_t)
            nc.vector.tensor_copy(out=hT[ct][:, ipt, :], in_=tp)
    hTm = [t.rearrange("p a b -> p (a b)") for t in hT]
    for ipt in range(2):
        fp = ps.tile([128, 4], F32, tag="mps", name="fps")
        for ct in range(2):
            nc.tensor.matmul(fp, lhsT=hTm[ct][:, ts(ipt, 128)],
                             rhs=owf_t[ct], start=(ct == 0), stop=(ct == 1))
        ot = sb.tile([128, 4], F32, tag="ot", name="ot")
        nc.vector.tensor_add(out=ot, in0=fp, in1=xskip_t[ipt])
        dma(out.ap()[ts(ipt, 128), :], ot)
    ctx.close()


# ------------------------------------------------------------------
# host side
# ------------------------------------------------------------------
_CACHE = {}


def _gelu_np(x):
    try:
        from scipy.special import erf
        e = erf(x / math.sqrt(2.0))
    except Exception:
        e = np.vectorize(math.erf)(x / math.sqrt(2.0))
    return 0.5 * x * (1.0 + e)


def _host_prep(inputs):
    f32 = np.float32
    sigma = float(inputs["sigma"])
    sd = SIGMA_DATA
    s2 = sigma * sigma + sd * sd
    c_skip = f32(sd * sd / s2)
    c_out = f32(sigma * sd / math.sqrt(s2))
    c_in = f32(1.0 / math.sqrt(s2))
    c_noise = f32(0.25 * math.log(sigma + 1e-8))

    half = CA // 2
    freqs = np.exp(-math.log(10000.0)
                   * np.arange(half, dtype=f32) / half).astype(f32)
    a = c_noise * freqs
    temb = np.concatenate([np.cos(a), np.sin(a)]).astype(f32)
    t1 = _gelu_np((temb @ inputs["tmlp_W1"].T
                   + inputs["tmlp_b1"]).astype(np.float64))
    tc_vec = (t1 @ inputs["tmlp_W2"].T.astype(np.float64)
              + inputs["tmlp_b2"]).astype(f32)

    lnA = np.zeros((NB, 2, CA), f32)
    lnB = np.zeros((NB, 2, CA), f32)
    for b in range(NB):
        for a_i, (g, bb, pW, pb) in enumerate([
            (inputs["ada1_g"][b], inputs["ada1_b"][b],
             inputs["ada1_pW"][b], inputs["ada1_pb"][b]),
            (inputs["ada2_g"][b], inputs["ada2_b"][b],
             inputs["ada2_pW"][b], inputs["ada2_pb"][b]),
        ]):
            ss = tc_vec @ pW.T + pb
            scale, shift = ss[:CA], ss[CA:]
            lnA[b, a_i] = g * (1.0 + scale)
            lnB[b, a_i] = bb * (1.0 + scale) + shift

    vw_aug = np.zeros((NB, VA, CA), f32)
    for hh in range(H):
        vw_aug[:, hh * (D + 1):hh * (D + 1) + D, :] = \
            inputs["vW"][:, hh * D:(hh + 1) * D, :]

    wall = (math.sqrt(D) * inputs["pairW"].reshape(NB * H, CZ)).astype(f32)
    pwbd = np.zeros((128, CZ), f32)
    pwbd[0:CZ, 0:32] = wall.T
    pwbd[CZ:128, 32:64] = wall.T

    pair_bf = np.ascontiguousarray(inputs["pair"]).astype(bf16)
    pair_bf = pair_bf.reshape(NCORES, R, 2, 512, 128)

    x = inputs["x_noisy"].astype(f32)
    xskip = np.zeros((L, 4), f32)
    xskip[:, 0:3] = c_skip * x + c_out * inputs["out_b"][None, :]
    owf = np.zeros((CA, 4), f32)
    owf[:, 0:3] = c_out * inputs["out_W"].T

    shared = {
        "coordwt": np.ascontiguousarray(inputs["coord_W"].T).astype(bf16),
        "snglwt": np.ascontiguousarray(inputs["single_W"].T).astype(bf16),
        "h0brow": (inputs["coord_b"]
                   + inputs["single_b"]).reshape(1, CA).astype(bf16),
        "qwt": np.ascontiguousarray(inputs["qW"].transpose(0, 2, 1)).astype(bf16),
        "kwt": np.ascontiguousarray(inputs["kW"].transpose(0, 2, 1)).astype(bf16),
        "vwt": np.ascontiguousarray(vw_aug.transpose(0, 2, 1)).astype(bf16),
        "owt": np.ascontiguousarray(inputs["outW"].transpose(0, 2, 1)).astype(bf16),
        "w1t": np.ascontiguousarray(inputs["ffn_W1"].transpose(0, 2, 1)).astype(bf16),
        "w2t": np.ascontiguousarray(inputs["ffn_W2"].transpose(0, 2, 1)).astype(bf16),
        "brows": np.concatenate(
            [inputs["outb"], inputs["ffn_b1"], inputs["ffn_b2"]],
            axis=-1).reshape(NB, 1, CA + FF + CA).astype(bf16),
        "pwbd": pwbd.astype(bf16),
        "lnA": lnA, "lnB": lnB, "owf": owf,
        "identb": np.eye(128, dtype=f32).astype(bf16),
        "identf": np.eye(128, dtype=f32),
    }
    xct = np.ascontiguousarray((c_in * x).T).astype(bf16)
    sngl = inputs["single"].astype(f32)

    in_maps = []
    for c in range(NCORES):
        rows = slice(c * R, (c + 1) * R)
        m = dict(shared)
        m["pairb"] = pair_bf[c]
        m["xsct"] = np.ascontiguousarray(xct[:, rows])
        m["xskip"] = np.ascontiguousarray(xskip[rows])
        m["snglt"] = np.ascontiguousarray(sngl[rows].T).astype(bf16)
        in_maps.append(m)
    return in_maps


def _get_nc():
    if "nc" not in _CACHE:
        _CACHE["nc"] = _build_nc()
    return _CACHE["nc"]


def _get_exec():
    """Cached jitted 8-core executor (mirrors bass2jax.run_bass_via_pjrt)."""
    if "exec" in _CACHE:
        return _CACHE["exec"]
    import jax
    import jax.numpy as jnp
    from jax.experimental.shard_map import shard_map
    from jax.sharding import Mesh, PartitionSpec
    from concourse.bass2jax import (_bass_exec_p, install_neuronx_cc_hook,
                                    partition_id_tensor)
    import concourse.mybir as mb

    nc = _get_nc()
    install_neuronx_cc_hook()
    pname = nc.partition_id_tensor.name if nc.partition_id_tensor else None
    in_names, out_names, out_avals, zero_shapes = [], [], [], []
    for alloc in nc.m.functions[0].allocations:
        if not isinstance(alloc, mb.MemoryLocationSet):
            continue
        name = alloc.memorylocations[0].name
        if alloc.kind == "ExternalInput":
            if name != pname:
                in_names.append(name)
        elif alloc.kind == "ExternalOutput":
            shape = tuple(alloc.tensor_shape)
            dtype = mb.dt.np(alloc.dtype)
            out_names.append(name)
            out_avals.append(jax.core.ShapedArray(shape, dtype))
            zero_shapes.append((shape, dtype))
    n_params = len(in_names)
    all_names = in_names + out_names
    if pname is not None:
        all_names = all_names + [pname]
    donate = tuple(range(n_params, n_params + len(out_names)))

    def _bodyfn(*args):
        operands = list(args)
        if pname is not None:
            operands.append(partition_id_tensor())
        outs = _bass_exec_p.bind(
            *operands, out_avals=tuple(out_avals), in_names=tuple(all_names),
            out_names=tuple(out_names), lowering_input_output_aliases=(),
            sim_require_finite=True, sim_require_nnan=True, nc=nc)
        return tuple(outs)

    devices = jax.devices()[:NCORES]
    mesh = Mesh(np.asarray(devices), ("core",))
    specs = (PartitionSpec("core"),) * (n_params + len(out_names))
    sharded = jax.jit(
        shard_map(_bodyfn, mesh=mesh, in_specs=specs,
                  out_specs=(PartitionSpec("core"),) * len(out_names),
                  check_rep=False),
        donate_argnums=donate, keep_unused=True)
    _CACHE["exec"] = dict(fn=sharded, in_names=in_names, out_names=out_names,
                          zero_shapes=zero_shapes, mesh=mesh)
    return _CACHE["exec"]


def _run(in_maps):
    ex = _get_exec()
    concat_in = [np.concatenate([np.asarray(m[n]) for m in in_maps], axis=0)
                 for n in ex["in_names"]]
    zeros = [np.zeros((NCORES * s[0], *s[1:]), d) for s, d in ex["zero_shapes"]]
    outs = ex["fn"](*concat_in, *zeros)
    return outs


def kernel(**inputs):
    inputs = {k: np.asarray(v) for k, v in inputs.items()}
    in_maps = _host_prep(inputs)
    outs = _run(in_maps)
    oi = _get_exec()["out_names"].index("out")
    full = np.asarray(outs[oi]).reshape(NCORES, R, 4)
    return np.ascontiguousarray(
        full[:, :, 0:3].reshape(L, 3)).astype(np.float32)


def bench(in_maps, iters=10):
    """Wall-clock the cached executor with device-resident inputs."""
    import time
    import jax
    from jax.sharding import NamedSharding, PartitionSpec
    ex = _get_exec()
    sh = NamedSharding(ex["mesh"], PartitionSpec("core"))
    concat_in = [jax.device_put(
        np.concatenate([np.asarray(m[n]) for m in in_maps], axis=0), sh)
        for n in ex["in_names"]]
    for a in concat_in:
        a.block_until_ready()
    times = []
    for _ in range(iters):
        zeros = [jax.device_put(np.zeros((NCORES * s[0], *s[1:]), d), sh)
                 for s, d in ex["zero_shapes"]]
        for z in zeros:
            z.block_until_ready()
        t0 = time.perf_counter()
        outs = ex["fn"](*concat_in, *zeros)
        for o in outs:
            o.block_until_ready()
        times.append(time.perf_counter() - t0)
    return times, outs


def bench_slope(in_maps, n_small=2, n_big=22):
    """Marginal per-dispatch time: fire N async dispatches, block once.
    slope = (t_big - t_small) / (n_big - n_small) ~= device time + per-call
    axon marshaling (~0.5 ms floor)."""
    import time
    import jax
    from jax.sharding import NamedSharding, PartitionSpec
    ex = _get_exec()
    sh = NamedSharding(ex["mesh"], PartitionSpec("core"))
    concat_in = [jax.device_put(
        np.concatenate([np.asarray(m[n]) for m in in_maps], axis=0), sh)
        for n in ex["in_names"]]
    for a in concat_in:
        a.block_until_ready()

    def run_n(n):
        zs = [[jax.device_put(np.zeros((NCORES * s[0], *s[1:]), d), sh)
               for s, d in ex["zero_shapes"]] for _ in range(n)]
        for z in zs:
            for x in z:
                x.block_until_ready()
        t0 = time.perf_counter()
        outs = None
        for i in range(n):
            outs = ex["fn"](*concat_in, *zs[i])
        for o in outs:
            o.block_until_ready()
        return time.perf_counter() - t0

    run_n(2)  # warm
    ts = run_n(n_small)
    tb = run_n(n_big)
    return ts, tb, (tb - ts) / (n_big - n_small)


if __name__ == "__main__":
    import reference
    ins = {k: np.asarray(v) for k, v in reference.setup_inputs().items()}
    got = kernel(**ins)
    want = np.asarray(reference.reference(**reference.setup_inputs()))
    rel = np.linalg.norm(got - want) / np.linalg.norm(want)
    print("max abs err", np.abs(got - want).max(), "rel l2", rel)
